# revision 40
# baseline (speedup 1.0000x reference)
"""Trainium2 Bass kernel for the SelfAttentionBlock problem (8 NeuronCores).

Sharding (same as baseline): MLP data-parallel over rows (each core owns
512 rows), AllToAll per tensor to switch to head-parallel attention
(core c computes head c for both batches), host concat of outputs.

v2 rewrite, driven by the baseline trace (305us):
  * ~95 descriptor-heavy DMAs -> ~20 dense ones; all weights/consts are
    host-packed into the exact SBUF layouts so every load is one
    contiguous [128, X] transfer.
  * AllToAll triggers fire right after each MLP section (they were all
    serialized after the MLP in the baseline, costing a 74us PE hole).
  * recv-side layouts redesigned so the attention input loads are three
    dense DMAs (the gpsimd vaug gather with 128B descriptors is gone).
  * q/k for both batches live in partitions 0-63 / 64-127, so the two
    batches' score matmuls (K=64 each) run concurrently in the PE array
    (row tiling via base_partition).
  * exp is batched 2 score tiles per ACTIVATE ([128,1024] PSUM groups).
  * scores for the late q-chunks are emitted inside/after the v-MLP so
    the ACT exp stream starts early and the v AllToAll is hidden.
  * softmax denominator reciprocal: DVE reciprocal_approx_fast (~5x
    faster than nc.vector.reciprocal, which cost 3.3us per chunk).
  * attention output stored bf16 ([2,64,2048] per core); the softmax
    denominator row is broadcast to 64 partitions via gpsimd
    partition_broadcast and divided out on DVE.
  * q-MLP runs first so its AllToAll (which gates the exp stream) fires
    earliest; a tiny warm-up collective absorbs the ~30us ncfw setup.
"""

import ml_dtypes
import numpy as np

import concourse.bass as bass
import concourse.mybir as mybir
from concourse import bacc, tile
from concourse.bass_utils import run_bass_kernel_spmd

N_CORES = 8
N, S, D, H = 2, 2048, 512, 2048
HEADS = 8
Dh = D // HEADS            # 64
RPC = S // N_CORES         # 256 rows per core per batch
ROWS = N * RPC             # 512 rows per core
KT1 = D // 128             # 4 layer-1 contraction tiles
MT1 = H // 128             # 16 layer-1 out tiles == layer-2 contraction tiles
MT2 = D // 128             # 4 layer-2 out tiles
QC = S // 512              # 4 q-chunks per batch

F32 = mybir.dt.float32
BF16 = mybir.dt.bfloat16
AF = mybir.ActivationFunctionType

# const blob column offsets
CF_B1 = {"k": 0, "q": 16, "v": 32}
CF_B2 = {"k": 48, "q": 52}
CF_ONES64 = 56             # f32 ones, row 0 used as [1,64] bcast lhsT
CF_W = 120
CB_BAND = 0                # [128, 1024] causal band
CB_B2V = 1024              # row 0: v second-layer bias [512]
CB_W = 1536


def _build():
    nc = bacc.Bacc("TRN2", target_bir_lowering=False, debug=False,
                   num_devices=N_CORES)

    xt_d = nc.dram_tensor("xt", [128, KT1 * ROWS], BF16, kind="ExternalInput")
    w1_d = {t: nc.dram_tensor(f"w1{t}", [128, MT1 * KT1 * 128], BF16,
                              kind="ExternalInput") for t in "kqv"}
    w2_d = {t: nc.dram_tensor(f"w2{t}", [128, MT1 * D], BF16,
                              kind="ExternalInput") for t in "kqv"}
    cf32_d = nc.dram_tensor("cf32", [128, CF_W], F32, kind="ExternalInput")
    cbf_d = nc.dram_tensor("cbf", [128, CB_W], BF16, kind="ExternalInput")

    kT_out = nc.dram_tensor("kT_out", [HEADS, Dh, ROWS], BF16,
                            kind="ExternalOutput")
    v_out = nc.dram_tensor("v_out", [128, HEADS * MT2 * Dh], BF16,
                           kind="ExternalOutput")
    aT_out = nc.dram_tensor("attn_outT", [N, Dh, S], BF16,
                            kind="ExternalOutput")

    with tile.TileContext(nc) as tc:
        with (
            tc.tile_pool(name="dram", bufs=1, space="DRAM") as dp,
            tc.tile_pool(name="cst", bufs=1) as cst,
            tc.tile_pool(name="w1p", bufs=2) as w1p,
            tc.tile_pool(name="w2p", bufs=2) as w2p,
            tc.tile_pool(name="h1p", bufs=20) as h1p,
            tc.tile_pool(name="l2p", bufs=2) as l2p,
            tc.tile_pool(name="exp", bufs=28) as expp,
            tc.tile_pool(name="sm", bufs=3) as smp,
            tc.tile_pool(name="ps", bufs=2, space="PSUM") as psp,
            tc.tile_pool(name="po", bufs=4, space="PSUM") as pop,
        ):
            send = {
                "k": dp.tile([HEADS, Dh, ROWS], BF16, tag="send_k",
                             name="send_k"),
                "q": dp.tile([HEADS, Dh, ROWS], BF16, tag="send_q",
                             name="send_q"),
                "v": dp.tile([HEADS, 128, MT2 * Dh], BF16, tag="send_v",
                             name="send_v"),
            }
            warm_s = dp.tile([HEADS, 64], BF16, tag="warm_s", name="warm_s")
            warm_r = dp.tile([HEADS, 64], BF16, tag="warm_r", name="warm_r")
            recv = {
                "k": dp.tile([HEADS, Dh, ROWS], BF16, tag="recv_k",
                             name="recv_k"),
                "q": dp.tile([HEADS, Dh, ROWS], BF16, tag="recv_q",
                             name="recv_q"),
                "v": dp.tile([HEADS, 128, MT2, Dh], BF16, tag="recv_v",
                             name="recv_v"),
            }

            # warm up the collectives firmware: absorbs the jittery first-
            # mesh floor AND acts as a cross-core sync that compresses the
            # peer-skew of the real collectives.
            nc.gpsimd.collective_compute(
                "AllToAll", mybir.AluOpType.bypass,
                replica_groups=[list(range(N_CORES))],
                ins=[warm_s.opt()], outs=[warm_r.opt()])

            # ---- persistent SBUF tiles / early DMAs ----
            xt = cst.tile([128, KT1 * ROWS], BF16, tag="xt")
            nc.sync.dma_start(xt[:], xt_d[:])
            w1sb = {"q": w1p.tile([128, MT1 * KT1 * 128], BF16, tag="w1",
                                  name="w1_q")}
            # split first weight load so L1-q can start sooner
            nc.sync.dma_start(w1sb["q"][:, 0:2048], w1_d["q"][:, 0:2048])
            cf32 = cst.tile([128, CF_W], F32, tag="cf32")
            nc.sync.dma_start(cf32[:], cf32_d[:])
            nc.sync.dma_start(w1sb["q"][:, 2048:4096], w1_d["q"][:, 2048:4096])
            nc.sync.dma_start(w1sb["q"][:, 4096:8192], w1_d["q"][:, 4096:8192])
            w2sb = {"q": w2p.tile([128, MT1 * D], BF16, tag="w2", name="w2_q")}
            nc.sync.dma_start(w2sb["q"][:], w2_d["q"][:])
            cbf = cst.tile([128, CB_W], BF16, tag="cbf")
            nc.sync.dma_start(cbf[:], cbf_d[:])

            ones_row = cst.tile([1, 128], BF16, tag="ones_row")
            nc.vector.memset(ones_row[:], 1.0)
            # attention input tiles: both batches stacked on partitions for
            # q/k (b=0 -> partitions 0:64, b=1 -> 64:128)
            kT2 = cst.tile([128, HEADS * RPC], BF16, tag="kT2")
            qT2 = cst.tile([128, HEADS * RPC], BF16, tag="qT2")
            vaug = cst.tile([128, HEADS * N * 2 * 65], BF16, tag="vaug")
            with nc.allow_low_precision(reason="ones fill"):
                nc.vector.memset(
                    vaug[:].rearrange("p (g e) -> p g e", e=65)[:, :, 64:65],
                    1.0)
            oT_all = cst.tile([Dh, N * S], BF16, tag="oT")

            def mlp(t, after_l1_m0=None, after_l1=None, after_l2_mid=None):
                h1_t = []
                for m in range(MT1):
                    pp = psp.tile([128, 512], F32, tag="sc", name=f"p1{t}{m}")
                    for kt in range(KT1):
                        nc.tensor.matmul(
                            pp[:],
                            w1sb[t][:, m * 512 + kt * 128:m * 512 +
                                    (kt + 1) * 128],
                            xt[:, kt * ROWS:(kt + 1) * ROWS],
                            start=(kt == 0), stop=(kt == KT1 - 1))
                    h1 = h1p.tile([128, ROWS], BF16, tag="h1",
                                  name=f"h1{t}{m}")
                    nc.scalar.activation(h1[:], pp[:], AF.Gelu_apprx_tanh,
                                         bias=cf32[:, CF_B1[t] + m:
                                                   CF_B1[t] + m + 1])
                    h1_t.append(h1)
                    if m == 0 and after_l1_m0:
                        after_l1_m0()
                if after_l1:
                    after_l1()
                l2 = l2p.tile([128, MT2 * 512], BF16, tag="l2", name=f"l2{t}")
                for mo in range(MT2):
                    pp = psp.tile([128, 512], F32, tag="sc", name=f"p2{t}{mo}")
                    if t != "v":    # out = W2^T h1T + b2, transposed [D, ROWS]
                        for kt in range(MT1):
                            nc.tensor.matmul(
                                pp[:],
                                w2sb[t][:, kt * D + mo * 128:
                                        kt * D + (mo + 1) * 128],
                                h1_t[kt][:],
                                start=(kt == 0), stop=(kt == MT1 - 1))
                        with nc.allow_low_precision(reason="bf16 out"):
                            nc.vector.tensor_scalar_add(
                                l2[:, mo * 512:(mo + 1) * 512], pp[:],
                                cf32[:, CF_B2[t] + mo:CF_B2[t] + mo + 1])
                    else:           # v: out = gelu(h1 W2 + b2), natural
                        for kt in range(MT1):
                            nc.tensor.matmul(
                                pp[:],
                                h1_t[kt][:, mo * 128:(mo + 1) * 128],
                                w2sb[t][:, kt * D:(kt + 1) * D],
                                start=(kt == 0), stop=False)
                        nc.tensor.matmul(pp[:], ones_row[:],
                                         cbf[0:1, CB_B2V:CB_B2V + 512],
                                         start=False, stop=True)
                        nc.scalar.activation(
                            l2[:].rearrange("p (c m d) -> p c m d", c=HEADS,
                                            m=MT2)[:, :, mo, :],
                            pp[:], AF.Gelu_apprx_tanh)
                    if mo == 1 and after_l2_mid:
                        after_l2_mid()
                return l2

            def a2a(t):
                nc.gpsimd.collective_compute(
                    "AllToAll", mybir.AluOpType.bypass,
                    replica_groups=[list(range(N_CORES))],
                    ins=[send[t].opt()], outs=[recv[t].opt()])

            def load_kw():
                w1sb["k"] = w1p.tile([128, MT1 * KT1 * 128], BF16, tag="w1",
                                     name="w1_k")
                nc.sync.dma_start(w1sb["k"][:], w1_d["k"][:])
                w2sb["k"] = w2p.tile([128, MT1 * D], BF16, tag="w2",
                                     name="w2_k")
                nc.sync.dma_start(w2sb["k"][:], w2_d["k"][:])

            def load_vw():
                w1sb["v"] = w1p.tile([128, MT1 * KT1 * 128], BF16, tag="w1",
                                     name="w1_v")
                nc.sync.dma_start(w1sb["v"][:], w1_d["v"][:])
                w2sb["v"] = w2p.tile([128, MT1 * D], BF16, tag="w2",
                                     name="w2_v")
                nc.sync.dma_start(w2sb["v"][:], w2_d["v"][:])

            # ---------------- MLP q (first: its a2a gates the exps) ----
            l2q = mlp("q", after_l1_m0=load_kw)
            nc.sync.dma_start(
                send["q"][:].rearrange("(m m2) p r -> (m2 p) m r", m2=2),
                l2q[:].rearrange("p (m r) -> p m r", m=MT2))
            load_vw()
            a2a("q")

            # ---------------- MLP k ----------------
            l2k = mlp("k")
            nc.sync.dma_start(
                send["k"][:].rearrange("(m m2) p r -> (m2 p) m r", m2=2),
                l2k[:].rearrange("p (m r) -> p m r", m=MT2))
            nc.sync.dma_start(
                kT_out.ap().rearrange("(m m2) p r -> (m2 p) m r", m2=2),
                l2k[:].rearrange("p (m r) -> p m r", m=MT2))
            a2a("k")

            # ---------------- attention helpers ----------------
            exps = {}

            def scores(qc):
                """Score matmuls + exp for both batches of chunk qc.

                b=0 lives in partitions 0:64, b=1 in 64:128; emitting the
                two batches' K=64 matmuls adjacently into different PSUM
                banks lets the PE run them concurrently (row tiling)."""
                nk = 4 * qc + 4
                exps[(0, qc)] = []
                exps[(1, qc)] = []
                for g in range(nk // 2):
                    pps = [psp.tile([128, 1024], F32, tag="sc",
                                    name=f"sc{b}{qc}{g}") for b in range(N)]
                    for h in range(2):
                        kt = 2 * g + h
                        for b in range(N):
                            nc.tensor.matmul(
                                pps[b][:, h * 512:(h + 1) * 512],
                                kT2[b * 64:(b + 1) * 64,
                                    kt * 128:(kt + 1) * 128],
                                qT2[b * 64:(b + 1) * 64,
                                    qc * 512:(qc + 1) * 512],
                                start=True, stop=True)
                    for b in range(N):
                        ex = expp.tile([128, 1024], BF16, tag="exp",
                                       name=f"ex{b}{qc}{g}")
                        nc.scalar.activation(ex[:], pps[b][:], AF.Exp,
                                             scale=0.125)
                        for h in range(2):
                            kt = 2 * g + h
                            o = kt * 128 - qc * 512
                            if o >= 0:   # diagonal tile: causal band mask
                                with nc.allow_low_precision(reason="mask"):
                                    nc.vector.tensor_mul(
                                        ex[:, h * 512:(h + 1) * 512],
                                        ex[:, h * 512:(h + 1) * 512],
                                        cbf[:, CB_BAND + 512 - o:
                                            CB_BAND + 1024 - o])
                        exps[(b, qc)].append(ex)

            po = {}

            def pv(b, qc):
                nk = 4 * qc + 4
                p = pop.tile([65, 512], F32, tag="po", name=f"po{b}{qc}")
                po[(b, qc)] = p
                for kt in range(nk):
                    g, h = kt // 2, kt % 2
                    j, h2 = kt // 2, kt % 2
                    gidx = j * 4 + b * 2 + h2
                    nc.tensor.matmul(
                        p[:],
                        vaug[:, gidx * 65:(gidx + 1) * 65],
                        exps[(b, qc)][g][:, h * 512:(h + 1) * 512],
                        start=(kt == 0), stop=(kt == nk - 1))

            def norm(b, qc):
                p = po[(b, qc)]
                d_sb = smp.tile([1, 512], F32, tag="d", name=f"d{b}{qc}")
                nc.vector.tensor_copy(d_sb[:], p[64:65, :])
                r = smp.tile([1, 512], F32, tag="r", name=f"r{b}{qc}")
                with nc.allow_low_precision(reason="approx recip"):
                    nc.vector.reciprocal_approx_fast(r[:], d_sb[:])
                rb = smp.tile([64, 512], F32, tag="rb", name=f"rb{b}{qc}")
                nc.gpsimd.partition_broadcast(rb[:], r[:], channels=64)
                with nc.allow_low_precision(reason="bf16"):
                    nc.vector.tensor_mul(
                        oT_all[:, b * S + qc * 512:b * S + (qc + 1) * 512],
                        p[0:64, :], rb[:])
                nc.sync.dma_start(
                    aT_out[b, :, qc * 512:(qc + 1) * 512],
                    oT_all[:, b * S + qc * 512:b * S + (qc + 1) * 512])

            # ---------------- MLP v ----------------
            l2v = mlp("v")
            nc.sync.dma_start(v_out.ap(), l2v[:])
            nc.sync.dma_start(
                send["v"][:].rearrange("c p md -> p c md"),
                l2v[:].rearrange("p (c md) -> p c md", c=HEADS))
            a2a("v")
            # recv-side loads; after all sends in the Sync queue so their
            # collective-completion waits cannot block a send (deadlock).
            for b in range(N):
                nc.sync.dma_start(
                    qT2[b * 64:(b + 1) * 64, :]
                    .rearrange("p (j r) -> p j r", j=HEADS),
                    recv["q"][:, :, b * RPC:(b + 1) * RPC]
                    .rearrange("j p r -> p j r"))
                nc.sync.dma_start(
                    kT2[b * 64:(b + 1) * 64, :]
                    .rearrange("p (j r) -> p j r", j=HEADS),
                    recv["k"][:, :, b * RPC:(b + 1) * RPC]
                    .rearrange("j p r -> p j r"))
            for b in range(N):
                for h in range(2):
                    nc.sync.dma_start(
                        vaug[:].rearrange("p (j b h e) -> p j b h e",
                                          j=HEADS, b=N, h=2)
                        [:, :, b, h, 0:64],
                        recv["v"][:, :, b * 2 + h, :]
                        .rearrange("j p d -> p j d"))

            # ---------------- attention main ----------------
            scores(3)
            scores(2)
            pv(0, 3)
            pv(1, 3)
            norm(0, 3)
            norm(1, 3)
            scores(1)
            scores(0)
            pv(0, 2)
            pv(1, 2)
            norm(0, 2)
            norm(1, 2)
            pv(0, 1)
            pv(1, 1)
            norm(0, 1)
            norm(1, 1)
            pv(0, 0)
            pv(1, 0)
            norm(0, 0)
            norm(1, 0)



    nc.compile()
    return nc


_COMPILED = None


def _get_compiled():
    global _COMPILED
    if _COMPILED is None:
        _COMPILED = _build()
    return _COMPILED


def _band_mask():
    return (np.arange(1024, dtype=np.int32)[None, :]
            >= (np.arange(128, dtype=np.int32)[:, None] + 512)).astype(
                np.float32)


def _bf16(a):
    return np.ascontiguousarray(np.asarray(a, dtype=np.float32)
                                .astype(ml_dtypes.bfloat16))


def _pack_w1(w):            # [512, 2048] -> [128, (m kt 128)]
    w = np.asarray(w, np.float32)
    return _bf16(w.reshape(KT1, 128, MT1, 128).transpose(1, 2, 0, 3)
                 .reshape(128, MT1 * KT1 * 128))


def _pack_w2(w):            # [2048, 512] -> [128, (kt d)]
    w = np.asarray(w, np.float32)
    return _bf16(w.reshape(MT1, 128, D).transpose(1, 0, 2)
                 .reshape(128, MT1 * D))


def _make_in_maps(x, qW1, qb1, qW2, qb2, kW1, kb1, kW2, kb2, vW1, vb1,
                  vW2, vb2):
    x = np.asarray(x, np.float32)
    cf32 = np.zeros((128, CF_W), np.float32)
    for t, b1 in (("k", kb1), ("q", qb1), ("v", vb1)):
        cf32[:, CF_B1[t]:CF_B1[t] + 16] = np.asarray(b1, np.float32) \
            .reshape(16, 128).T
    for t, b2 in (("k", kb2), ("q", qb2)):
        cf32[:, CF_B2[t]:CF_B2[t] + 4] = np.asarray(b2, np.float32) \
            .reshape(4, 128).T
    cf32[:, CF_ONES64:CF_ONES64 + 64] = 1.0
    cbf = np.zeros((128, CB_W), np.float32)
    cbf[:, CB_BAND:CB_BAND + 1024] = _band_mask()
    cbf[0, CB_B2V:CB_B2V + 512] = np.asarray(vb2, np.float32)
    shared = {
        "w1q": _pack_w1(qW1), "w1k": _pack_w1(kW1), "w1v": _pack_w1(vW1),
        "w2q": _pack_w2(qW2), "w2k": _pack_w2(kW2), "w2v": _pack_w2(vW2),
        "cf32": cf32, "cbf": cbf.astype(ml_dtypes.bfloat16),
    }
    in_maps = []
    for c in range(N_CORES):
        xc = np.concatenate([x[b, c * RPC:(c + 1) * RPC, :]
                             for b in range(N)], 0)       # [ROWS, D]
        xT = np.ascontiguousarray(xc.T)                   # [D, ROWS]
        im = dict(shared)
        im["xt"] = _bf16(xT.reshape(KT1, 128, ROWS).transpose(1, 0, 2)
                         .reshape(128, KT1 * ROWS))
        in_maps.append(im)
    return in_maps


def _assemble(res):
    k_full = np.empty((N, S, D), np.float32)
    v_full = np.empty((N, S, D), np.float32)
    out_full = np.empty((N, S, D), np.float32)
    for j in range(N_CORES):
        kT_j = np.asarray(res[j]["kT_out"], np.float32)   # [8, 64, ROWS]
        v_j = np.asarray(res[j]["v_out"], np.float32) \
            .reshape(128, HEADS, N, 2, Dh).transpose(2, 3, 0, 1, 4) \
            .reshape(N, RPC, D)                           # [N, RPC, D]
        aT_j = np.asarray(res[j]["attn_outT"], np.float32)  # [N, Dh, S]
        # kT_j[h, p, b*256+rr] = k[b, j*RPC+rr, h*64+p]
        kk = kT_j.reshape(HEADS, Dh, N, RPC).transpose(2, 3, 0, 1) \
            .reshape(N, RPC, D)
        for b in range(N):
            k_full[b, j * RPC:(j + 1) * RPC, :] = kk[b]
            v_full[b, j * RPC:(j + 1) * RPC, :] = v_j[b]
            out_full[b, :, j * Dh:(j + 1) * Dh] = aT_j[b].T
    return k_full, v_full, out_full


def kernel(**inputs):
    nc = _get_compiled()
    in_maps = _make_in_maps(**inputs)
    res = run_bass_kernel_spmd(nc, in_maps, list(range(N_CORES))).results
    return _assemble(res)
